# revision 34
# baseline (speedup 1.0000x reference)
"""BaseGCN (4-layer GCN + mean-pool + MLP) on 8 Trainium2 NeuronCores.

Strategy: dst-sharded graph parallel, GPSIMD ap_gather message gathering,
PE one-hot scatter matmuls with one-hots built ON-CHIP via gpsimd
local_scatter (48KB/bank stream instead of 1MB one-hot loads from HBM).

  - z tables live in SBUF feature-major: [128 partitions, QS] where a
    16-partition row group holds one src core's features.  Self-loop
    contributions are added algebraically (xself / z*dinv2 terms).
  - dst nodes are packed into "bins" (<= 62 nodes) such that every
    (bin, chunk) has <= 128 edges; a bin owns one 128-column block of the
    gather stream, shared by all 8 chunks.
  - GPSIMD ap_gather pulls per-edge columns feature-major; scalar engine
    casts to fp16; PE transposes 128-col slabs to edge-major; K=128 fp16
    matmuls against on-chip one-hot tiles accumulate agg [w, 496] per
    bank in PSUM.
  - One-hot tiles [128, 3968] fp16 are built per bank by two gpsimd
    local_scatter ops from a [128, 64] fp16 value + [128, 64] int16 index
    stream (per-edge norm values scattered to their dst columns).
  - Weights (fp16) / bias / PReLU applied in transposed space; z^T written
    per-bank to DRAM; AllGather (Shared outputs) exchanges z^T between
    layers.
  - Layer 4 aggregates z4 = h3 @ (W4 lw1 lw2) at width 4, then pools via
    per-bank transposes + matmuls against a static [slot, graph] 1/cnt
    matrix (self terms pooled from layer 3 via dinv2/cnt weights);
    AllReduce + constant finishes.

GCNConv(x) = A_hat (x W) + b with A_hat = D^-1/2 A D^-1/2 + D^-1 I;
aggregation commutes with the weight matmul so we aggregate at
width min(d_in, d_out): widths 8, 8, 32, 4.
"""

import os
import numpy as np

# ---------------- problem constants (hardcoded per the contract) ----------
N = 100000
E = 1600000
B = 64
NC = 8
NPC = N // NC          # 12500 dst nodes per core
CAP = 128              # edges per (bin, chunk) == columns per bin
BIN = 62               # node columns per bin
BANKW = 8 * BIN        # 496 agg/z columns per bank
HALF = 32 * BIN        # 1984 one-hot columns per local_scatter half
OHW = 64 * BIN         # 3968 one-hot columns per bank
NI8 = 1024             # gather columns per bank (8 bins x 128)
D_IN = 8
DIMS = [8, 32, 64]             # h widths for layers 1..3
AGG_W = [8, 8, 32, 4]          # aggregation widths per layer
F32 = np.float32


def _bin_nodes(sizes, maxn, group):
    """Balanced multiway packing: fix a target bin count, then place each
    node (desc by load) into the feasible bin minimizing the resulting max
    chunk load.  sizes: [n, CH] int.  Returns (bin_of, col_of, n_bins)."""
    n, ch = sizes.shape
    tot = sizes.sum(1)
    target = max(int(np.ceil(n / maxn)),
                 int(np.ceil(sizes.sum(0).max() * 1.035 / CAP)),
                 int(np.ceil(sizes.sum() * 1.03 / (ch * CAP))))
    target = int(np.ceil(target / group) * group)
    order = np.argsort(-tot, kind="stable")
    cap_bins = target + 4 * group
    loads = np.zeros((cap_bins, ch), np.int64)
    cnts = np.zeros(cap_bins, np.int64)
    nbins = target
    bin_of = np.zeros(n, np.int32)
    col_of = np.zeros(n, np.int32)
    for v in order:
        s = sizes[v]
        cand = loads[:nbins] + s
        mx = cand.max(1)
        feas = (cand <= CAP).all(1) & (cnts[:nbins] < maxn)
        if feas.any():
            mx[~feas] = 1 << 30
            bi = int(np.argmin(mx))
        else:
            bi = nbins
            nbins += 1
        bin_of[v] = bi
        col_of[v] = cnts[bi]
        loads[bi] += s
        cnts[bi] += 1
    return bin_of, col_of, nbins


def _preprocess(x, edge_index, batch):
    src = edge_index[0].astype(np.int64)
    dst = edge_index[1].astype(np.int64)
    batch = batch.astype(np.int64)

    deg = np.bincount(dst, minlength=N).astype(F32) + 1.0
    dinv = (1.0 / np.sqrt(deg)).astype(F32)
    dinv2 = (dinv * dinv).astype(F32)

    # edge-only streams (self loops handled via algebraic diagonal terms)
    allsrc = src
    alldst = dst
    allval = (dinv[src] * dinv[dst]).astype(F32)

    cnt = np.maximum(np.bincount(batch, minlength=B).astype(F32), 1.0)

    sc8 = (allsrc // NPC).astype(np.int64)            # src core = chunk id
    e_core = (alldst // NPC).astype(np.int64)

    # ---- per-core binning (one bin structure for all layers) ------------
    bin_of = np.zeros(N, np.int32); col_of = np.zeros(N, np.int32)
    nb8 = []
    for c in range(NC):
        lo, hi = c * NPC, (c + 1) * NPC
        me = (alldst >= lo) & (alldst < hi)
        d8 = np.zeros((NPC, 8), np.int64)
        np.add.at(d8, (alldst[me] - lo, sc8[me]), 1)
        b_o, c_o, nb = _bin_nodes(d8, BIN, 8)
        bin_of[lo:hi] = b_o; col_of[lo:hi] = c_o; nb8.append(nb)
    NQB = int(np.ceil(max(nb8) / 8) * 8)         # bins count (x8 per bank)
    B8 = NQB // 8
    QS = NQB * BIN                               # z slots per core
    assert QS <= 16384, QS

    qs_slot = (bin_of.astype(np.int64) * BIN + col_of)   # core-local

    idx8 = np.zeros((NC, B8, 128, NI8 // 16), np.int16)
    # one-hots partition-major: [128, B8*OHW] so a G-bank group loads as
    # 128 large contiguous descriptors (descriptor-rate limited DMA)
    ohT = np.zeros((NC, 128, B8 * OHW), np.float16)

    for c in range(NC):
        m = (e_core == c)
        ev = allval[m]
        esrc = allsrc[m]
        edst = alldst[m]
        esc8 = sc8[m]
        key = bin_of[edst].astype(np.int64) * 8 + esc8
        order = np.argsort(key, kind="stable")
        ks = key[order]
        starts = np.searchsorted(ks, np.arange(NQB * 8))
        pos = np.arange(len(ks)) - starts[ks]
        assert pos.max() < CAP
        bn, ch = ks // 8, ks % 8
        col = bn * CAP + pos
        bank, cb = col // NI8, col % NI8
        idx8[c, bank, 16 * ch + cb % 16, cb // 16] = qs_slot[esrc[order]]
        blk = cb // 128                       # bin within bank (0..7)
        p = cb % 128                          # edge position = partition
        ohT[c, p, bank * OHW + (blk * 8 + ch) * BIN + col_of[edst[order]]] = \
            ev[order]

    # layer-4 pooling one-hots in QS space: [B8 banks, 128, 4, B] fp16
    poolw = np.zeros((NC, B8, 128, 4, B), np.float16)    # 1/cnt
    pool2w = np.zeros((NC, B8, 128, 4, B), np.float16)   # dinv2/cnt (selfs)
    # xself: x * dinv2 in QS columns (layer-1 diagonal term)
    xself = np.zeros((NC, 8, QS), F32)
    # dinv2 in QS columns (layer-2/3 diagonal terms); rows 0:16 and 32:48
    # carry dinv2 for the padded 48-row layer-3 layout, middle rows zero
    d2q = np.zeros((NC, 48, QS), F32)
    # x in table layout [128, QS]
    xtab = np.zeros((128, QS), F32)
    for c in range(NC):
        nodes = np.arange(c * NPC, (c + 1) * NPC)
        s = qs_slot[nodes]
        g = batch[nodes]
        jb = (s % BANKW) // 128
        pp = (s % BANKW) % 128
        poolw[c, s // BANKW, pp, jb, g] = (1.0 / cnt[g]).astype(np.float16)
        pool2w[c, s // BANKW, pp, jb, g] = \
            (dinv2[nodes] / cnt[g]).astype(np.float16)
        xself[c][:, s] = x[nodes].T * dinv2[nodes][None, :]
        d2q[c, :16][:, s] = dinv2[nodes][None, :].repeat(16, 0)
        d2q[c, 32:48][:, s] = dinv2[nodes][None, :].repeat(16, 0)
        xtab[16 * c:16 * c + D_IN, s] = x[nodes].T

    cfg = dict(B8=B8, QS=QS)
    return cfg, xtab, idx8, ohT, poolw, pool2w, xself, d2q


def _build_program(cfg):
    import concourse.bacc as bacc
    import concourse.tile as tile
    import concourse.mybir as mybir
    from concourse.masks import make_identity
    from contextlib import ExitStack

    dt = mybir.dt
    B8, QS = cfg["B8"], cfg["QS"]

    nc = bacc.Bacc("TRN2", target_bir_lowering=False, debug=False, num_devices=NC)

    xtab_d = nc.dram_tensor("xtab", [128, QS], dt.float32, kind="ExternalInput")
    idx8_d = nc.dram_tensor("idx8", [B8, 128, NI8 // 16], dt.int16, kind="ExternalInput")
    ohT_d = nc.dram_tensor("ohT", [128, B8 * OHW], dt.float16, kind="ExternalInput")
    poolw_d = nc.dram_tensor("poolw", [B8, 128, 4, B], dt.float16,
                             kind="ExternalInput")
    pool2w_d = nc.dram_tensor("pool2w", [B8, 128, 4, B], dt.float16,
                              kind="ExternalInput")
    xself_d = nc.dram_tensor("xself", [8, QS], dt.float32, kind="ExternalInput")
    d2q_d = nc.dram_tensor("d2q", [48, QS], dt.float32, kind="ExternalInput")
    Wd = {}
    for i, (ki, ko) in enumerate([(8, 8), (8, 32), (48, 64)]):
        Wd[i] = nc.dram_tensor(f"W{i+1}", [ki, ko], dt.float16, kind="ExternalInput")
    Wd[3] = nc.dram_tensor("W4", [64, 4], dt.float32, kind="ExternalInput")
    bd, ad = {}, {}
    for i, d in enumerate(DIMS):
        bd[i] = nc.dram_tensor(f"b{i+1}", [d, 1], dt.float32, kind="ExternalInput")
        ad[i] = nc.dram_tensor(f"a{i+1}", [d, 1], dt.float32, kind="ExternalInput")
    cvec_d = nc.dram_tensor("cvec", [4, 1], dt.float32, kind="ExternalInput")
    out_d = nc.dram_tensor("out", [4, B], dt.float32, kind="ExternalOutput")

    AG = mybir.AluOpType

    with tile.TileContext(nc) as tc, ExitStack() as ctx:
        wpool = ctx.enter_context(tc.tile_pool(name="weights", bufs=1))
        dram = ctx.enter_context(tc.tile_pool(name="dram", bufs=1, space="DRAM"))
        sb = ctx.enter_context(tc.tile_pool(name="sb", bufs=3))
        sbB = ctx.enter_context(tc.tile_pool(name="sbB", bufs=2))
        psA = ctx.enter_context(tc.tile_pool(name="psA", bufs=2, space="PSUM"))
        psB = ctx.enter_context(tc.tile_pool(name="psB", bufs=1, space="PSUM"))
        psC = ctx.enter_context(tc.tile_pool(name="psC", bufs=1, space="PSUM"))
        psT = ctx.enter_context(tc.tile_pool(name="psT", bufs=2, space="PSUM"))
        psT2 = ctx.enter_context(tc.tile_pool(name="psT2", bufs=1, space="PSUM"))
        psP = ctx.enter_context(tc.tile_pool(name="psP", bufs=1, space="PSUM"))

        table = wpool.tile([128, 2 * QS], dt.float32, name="table")
        ident16 = wpool.tile([128, 128], dt.float16, name="ident16")
        make_identity(nc, ident16[:])

        Wt, bt, at = {}, {}, {}
        for i, (ki, ko) in enumerate([(8, 8), (8, 32), (48, 64)]):
            Wt[i] = wpool.tile([ki, ko], dt.float16, tag=f"w{i}", name=f"wt{i}")
            nc.sync.dma_start(Wt[i][:], Wd[i][:])
        Wt[3] = wpool.tile([64, 4], dt.float32, tag="w3", name="wt3")
        nc.sync.dma_start(Wt[3][:], Wd[3][:])
        for i, d in enumerate(DIMS):
            bt[i] = wpool.tile([d, 1], dt.float32, tag=f"b{i}", name=f"bt{i}")
            nc.sync.dma_start(bt[i][:], bd[i][:])
            at[i] = wpool.tile([d, 1], dt.float32, tag=f"a{i}", name=f"at{i}")
            nc.sync.dma_start(at[i][:], ad[i][:])
        cvt = wpool.tile([4, 1], dt.float32, name="cvt")
        nc.sync.dma_start(cvt[:], cvec_d[:])

        zownT = {1: dram.tile([8, QS], dt.float32, name="zo1"),
                 2: dram.tile([32, QS], dt.float32, name="zo2"),
                 3: dram.tile([4, QS], dt.float32, name="zo3")}
        zfullT = {1: dram.tile([NC, 8, QS], dt.float32, name="zf1",
                               addr_space="Shared"),
                  2: dram.tile([NC, 32, QS], dt.float32, name="zf2",
                               addr_space="Shared"),
                  3: dram.tile([NC, 4, QS], dt.float32, name="zf3",
                               addr_space="Shared")}
        pool_in = dram.tile([4, B], dt.float32, name="pin")
        pool_out = dram.tile([4, B], dt.float32, name="pout")

        GRP = 3                      # banks per one-hot DMA group

        def agg_phase(tab_offs, w, body, extra=None):
            """Gather + transpose + one-hot scatter matmuls for all banks.
            One-hots stream from DRAM in GRP-bank groups (128 large
            descriptors each).  body(bank, h, t, c, lhsT_ap, oh_ap) emits
            one K=128 matmul per (block, chunk)."""
            for g in range((B8 + GRP - 1) // GRP):
                lo = g * GRP
                n = min(GRP, B8 - lo)
                ohg = sb.tile([128, GRP * OHW], dt.float16, tag="ohg",
                              bufs=2, name="ohg")
                nc.sync.dma_start(ohg[:, 0:n * OHW],
                                  ohT_d[:, lo * OHW:(lo + n) * OHW])
                for s in range(lo, lo + n):
                    ob = (s - lo) * OHW
                    idx_t = sb.tile([128, NI8 // 16], dt.int16, tag="idx",
                                    name="idx")
                    nc.sync.dma_start(idx_t[:], idx8_d[s])
                    if extra is not None:
                        extra(s)
                    for h, off in enumerate(tab_offs):
                        msgT = sb.tile([128, NI8], dt.float32, tag="msg",
                                       bufs=2, name="msg")
                        nc.gpsimd.ap_gather(msgT[:], table[:, off:off + QS],
                                            idx_t[:], channels=128,
                                            num_elems=QS, d=1, num_idxs=NI8)
                        msg16 = sb.tile([128, NI8], dt.float16, tag="msg16",
                                        bufs=2, name="msg16")
                        nc.scalar.copy(msg16[:], msgT[:])
                        for sg in range(2):
                            trp = psT.tile([128, 512], dt.float16, tag="trp",
                                           name="trp")
                            for jp in range(4):
                                nc.tensor.transpose(
                                    trp[:, jp * 128:jp * 128 + 128],
                                    msg16[:, 128 * (sg * 4 + jp):128 * (sg * 4 + jp) + 128],
                                    ident16[:])
                            slabs = sbB.tile([128, 512], dt.float16,
                                             tag="slabs", name="slabs")
                            nc.scalar.copy(slabs[:], trp[:])
                            for jp in range(4):
                                t = sg * 4 + jp
                                for c in range(8):
                                    body(s, h, t, c,
                                         slabs[:, jp * 128 + 16 * c:jp * 128 + 16 * c + w],
                                         ohg[:, ob:ob + OHW])

        def layer(l):  # l = 0, 1, 2
            w = AGG_W[l]
            d = DIMS[l]
            if l == 0:
                nc.scalar.dma_start(table[:, 0:QS], xtab_d[:])
            elif l == 1:
                for c in range(NC):
                    nc.scalar.dma_start(table[16 * c:16 * c + 8, 0:QS],
                                        zfullT[1][c])
            else:
                for c in range(NC):
                    nc.scalar.dma_start(table[16 * c:16 * c + 16, 0:QS],
                                        zfullT[2][c, 0:16])
                    nc.scalar.dma_start(table[16 * c:16 * c + 16, QS:2 * QS],
                                        zfullT[2][c, 16:32])

            state = {}
            wh = 16 if l == 2 else w
            rows = 48 if l == 2 else w
            roff = 32 if l == 2 else 0          # PSUM row offset for half 1

            def extra(bank):
                """Prefetch the self-term operands on the sync queue."""
                sl = slice(BANKW * bank, BANKW * bank + BANKW)
                if l == 0:
                    xs = sb.tile([8, BANKW], dt.float32, tag="xs", bufs=2,
                                 name="xs")
                    nc.sync.dma_start(xs[:], xself_d[:, sl])
                    state["self"] = xs
                elif l == 1:
                    zs = sb.tile([8, BANKW], dt.float32, tag="xs", bufs=2,
                                 name="zs")
                    nc.sync.dma_start(zs[:], zownT[1][:, sl])
                    d2 = sb.tile([8, BANKW], dt.float32, tag="d2", bufs=2,
                                 name="d2")
                    nc.sync.dma_start(d2[:], d2q_d[0:8, sl])
                    state["self"] = (zs, d2)
                else:
                    zs = sb.tile([48, BANKW], dt.float32, tag="xs", bufs=2,
                                 name="zs3")
                    nc.sync.dma_start(zs[0:16, :], zownT[2][0:16, sl])
                    nc.sync.dma_start(zs[32:48, :], zownT[2][16:32, sl])
                    d2 = sb.tile([48, BANKW], dt.float32, tag="d2", bufs=2,
                                 name="d23")
                    nc.sync.dma_start(d2[:], d2q_d[:, sl])
                    state["self"] = (zs, d2)

            def body(bank, h, t, c, lhsT, oh_t):
                if h == 0 and t == 0 and c == 0:
                    state["agg"] = psA.tile([48, 512], dt.float32, tag="agg",
                                            name="agg")
                nc.tensor.matmul(
                    state["agg"][roff * h:roff * h + wh,
                                 BIN * t:BIN * t + BIN],
                    lhsT=lhsT,
                    rhs=oh_t[:, (t * 8 + c) * BIN:(t * 8 + c + 1) * BIN],
                    start=(c == 0), stop=(c == 7))
                if l == 2:
                    done = (h == 1 and t == 7 and c == 7)
                else:
                    done = (t == 7 and c == 7)
                if done:
                    bphase(bank, state["agg"])

            def bphase(bank, agg_ps):
                aggs = sbB.tile([rows, BANKW], dt.float16, tag="aggs",
                                name="aggs")
                if l == 0:
                    nc.vector.tensor_add(aggs[:], agg_ps[0:rows, 0:BANKW],
                                         state["self"][:])
                elif l == 1:
                    zs, d2 = state["self"]
                    zsd = sbB.tile([8, BANKW], dt.float32, tag="zsd", bufs=1,
                                   name="zsd")
                    nc.gpsimd.tensor_mul(zsd[:], zs[:], d2[:])
                    nc.vector.tensor_add(aggs[:], agg_ps[0:rows, 0:BANKW],
                                         zsd[:])
                else:
                    zs, d2 = state["self"]
                    zsd = sbB.tile([48, BANKW], dt.float32, tag="zsd3", bufs=1,
                                   name="zsd3")
                    nc.gpsimd.tensor_mul(zsd[0:16, :], zs[0:16, :], d2[0:16, :])
                    nc.gpsimd.tensor_mul(zsd[32:48, :], zs[32:48, :],
                                         d2[32:48, :])
                    nc.vector.memset(aggs[:], 0.0)
                    nc.vector.tensor_add(aggs[0:16, :], agg_ps[0:16, 0:BANKW],
                                         zsd[0:16, :])
                    nc.vector.tensor_add(aggs[32:48, :],
                                         agg_ps[32:48, 0:BANKW],
                                         zsd[32:48, :])
                h_ps = psB.tile([d, BANKW], dt.float32, tag="h", name="h")
                nc.tensor.matmul(h_ps[:], lhsT=Wt[l][:], rhs=aggs[:],
                                 start=True, stop=True)
                neg = sbB.tile([d, BANKW], dt.float32, tag="neg", name="neg")
                nc.vector.tensor_scalar(neg[:], h_ps[:], bt[l][:], 0.0, AG.add, AG.min)
                nega = sbB.tile([d, BANKW], dt.float32, tag="nega", name="nega")
                nc.vector.tensor_scalar(nega[:], neg[:], at[l][:], None, AG.mult)
                pos = sbB.tile([d, BANKW], dt.float32, tag="pos", name="pos")
                nc.vector.tensor_scalar(pos[:], h_ps[:], bt[l][:], 0.0, AG.add, AG.max)
                hT = sbB.tile([d, BANKW], dt.float32, tag="hT", name="hT")
                nc.vector.tensor_add(hT[:], pos[:], nega[:])
                if l == 2:
                    z4_ps = psC.tile([4, BANKW], dt.float32, tag="z4", name="z4")
                    nc.tensor.matmul(z4_ps[:], lhsT=Wt[3][:], rhs=hT[:],
                                     start=True, stop=True)
                    z4s = sbB.tile([4, BANKW], dt.float16, tag="z4s", name="z4s")
                    nc.scalar.copy(z4s[:], z4_ps[:])
                    z4f = sbB.tile([4, BANKW], dt.float32, tag="z4f", bufs=1,
                                   name="z4f")
                    nc.scalar.copy(z4f[:], z4_ps[:])
                    # store via SWDGE: keeps the slow HBM-write completion
                    # off the shared DMAHW semaphore lanes the loads use
                    nc.gpsimd.dma_start(zownT[3][:, BANKW * bank:BANKW * bank + BANKW],
                                        z4f[:])
                    # layer-4 self-loop term: pool dinv2*z4/cnt directly
                    p2 = state["p2"].pop(bank)
                    for j in range(4):
                        wdt = 128 if j < 3 else BANKW - 384
                        trp2 = psT2.tile([128, 4], dt.float16, tag="trp2",
                                         name="trp2")
                        nc.tensor.transpose(trp2[0:wdt, :],
                                            z4s[:, 128 * j:128 * j + wdt],
                                            ident16[:4, :4])
                        trs = sbB.tile([128, 4], dt.float16, tag="trs",
                                       name="trs2")
                        nc.scalar.copy(trs[0:wdt, :], trp2[0:wdt, :])
                        nc.tensor.matmul(pool_ps[:],
                                         lhsT=trs[0:wdt, :],
                                         rhs=p2[0:wdt, j, :],
                                         start=(bank == 0 and j == 0),
                                         stop=False)
                else:
                    nc.gpsimd.dma_start(
                        zownT[l + 1][:, BANKW * bank:BANKW * bank + BANKW], hT[:])

            if l == 2:
                state["p2"] = {}

                def extra2(bank):
                    extra(bank)
                    p2 = sb.tile([128, 4, B], dt.float16, tag="pw", bufs=2,
                                 name="p2w")
                    nc.sync.dma_start(p2[:], pool2w_d[bank])
                    state["p2"][bank] = p2

                agg_phase([0, QS], 16, body, extra=extra2)
            else:
                agg_phase([0], w, body, extra=extra)
            zkey = l + 1 if l < 2 else 3
            if os.environ.get("GCN_NO_CC"):
                nc.sync.dma_start(zfullT[zkey][0], zownT[zkey][:])
            else:
                nc.gpsimd.collective_compute(
                    "AllGather", AG.bypass, replica_groups=[list(range(NC))],
                    ins=[zownT[zkey][:].opt()], outs=[zfullT[zkey][:].opt()])

        pool_ps = psP.tile([4, B], dt.float32, name="pool_ps")

        for l in range(3):
            layer(l)

        # ---- layer 4: per-node agg of z4, then mean-pool ---------------
        for c in range(NC):
            nc.scalar.dma_start(table[16 * c:16 * c + 4, 0:QS], zfullT[3][c])
        state4 = {}
        pw_t = {}

        def extra4(bank):
            pw = sb.tile([128, 4, B], dt.float16, tag="pw", bufs=2, name="pw")
            nc.sync.dma_start(pw[:], poolw_d[bank])
            pw_t[bank] = pw

        def body4(bank, h, t, c, lhsT, oh_t):
            if t == 0 and c == 0:
                state4["agg"] = psA.tile([48, 512], dt.float32, tag="agg",
                                         name="agg4")
            nc.tensor.matmul(state4["agg"][0:4, BIN * t:BIN * t + BIN],
                             lhsT=lhsT,
                             rhs=oh_t[:, (t * 8 + c) * BIN:(t * 8 + c + 1) * BIN],
                             start=(c == 0), stop=(c == 7))
            if t == 7 and c == 7:
                pphase(bank, state4["agg"])

        def pphase(bank, agg_ps):
            a16 = sbB.tile([4, BANKW], dt.float16, tag="aggs", name="agg4s")
            nc.scalar.copy(a16[:], agg_ps[0:4, 0:BANKW])
            pw = pw_t.pop(bank)
            for j in range(4):
                wdt = 128 if j < 3 else BANKW - 384
                trp2 = psT2.tile([128, 4], dt.float16, tag="trp2", name="trp4")
                nc.tensor.transpose(trp2[0:wdt, :],
                                    a16[:, 128 * j:128 * j + wdt],
                                    ident16[:4, :4])
                trs = sbB.tile([128, 4], dt.float16, tag="trs", name="trs")
                nc.scalar.copy(trs[0:wdt, :], trp2[0:wdt, :])
                nc.tensor.matmul(pool_ps[:],
                                 lhsT=trs[0:wdt, :], rhs=pw[0:wdt, j, :],
                                 start=False,
                                 stop=(bank == B8 - 1 and j == 3))

        agg_phase([0], 4, body4, extra=extra4)

        pooled = sbB.tile([4, B], dt.float32, name="pooled")
        nc.scalar.copy(pooled[:], pool_ps[:])
        nc.sync.dma_start(pool_in[:], pooled[:])
        if os.environ.get("GCN_NO_CC"):
            nc.sync.dma_start(pool_out[:], pool_in[:])
        else:
            nc.gpsimd.collective_compute(
                "AllReduce", AG.add, replica_groups=[list(range(NC))],
                ins=[pool_in[:].opt()], outs=[pool_out[:].opt()])
        res = sbB.tile([4, B], dt.float32, name="res")
        nc.sync.dma_start(res[:], pool_out[:])
        res2 = sbB.tile([4, B], dt.float32, name="res2")
        nc.vector.tensor_scalar(res2[:], res[:], cvt[:], None, AG.add)
        nc.sync.dma_start(out_d[:], res2[:])

    nc.compile()
    return nc


def build(inputs):
    """Host preprocessing + program build. Returns (nc, in_maps)."""
    x = np.asarray(inputs["x"], F32)
    edge_index = np.asarray(inputs["edge_index"])
    batch = np.asarray(inputs["batch"])
    W = [np.asarray(inputs[f"W{i}"], F32) for i in range(1, 5)]
    b = [np.asarray(inputs[f"b{i}"], F32) for i in range(1, 5)]
    a = [np.asarray(inputs[f"a{i}"], F32) for i in range(1, 4)]
    lw1 = np.asarray(inputs["lw1"], F32)
    lb1 = np.asarray(inputs["lb1"], F32)
    lw2 = np.asarray(inputs["lw2"], F32)
    lb2 = np.asarray(inputs["lb2"], F32)

    (cfg, xtab, idx8, ohT, poolw, pool2w,
     xself, d2q) = _preprocess(x, edge_index, batch)

    W4p = (W[3] @ lw1 @ lw2).astype(F32)                     # [64, 4]
    W3p = np.zeros((48, 64), np.float16)                     # padded rows
    W3p[0:16] = W[2][0:16]
    W3p[32:48] = W[2][16:32]
    cv = (b[3] @ lw1 @ lw2 + lb1 @ lw2 + lb2).astype(F32)    # [4]

    nc = _build_program(cfg)

    in_maps = []
    for c in range(NC):
        m = dict(
            xtab=xtab, idx8=idx8[c], ohT=ohT[c],
            poolw=poolw[c], pool2w=pool2w[c], xself=xself[c], d2q=d2q[c],
            W1=W[0].astype(np.float16), W2=W[1].astype(np.float16),
            W3=W3p, W4=W4p,
            b1=b[0].reshape(-1, 1), b2=b[1].reshape(-1, 1), b3=b[2].reshape(-1, 1),
            a1=np.full((8, 1), a[0][0], F32),
            a2=np.full((32, 1), a[1][0], F32),
            a3=np.full((64, 1), a[2][0], F32),
            cvec=cv.reshape(4, 1),
        )
        in_maps.append(m)
    return nc, in_maps


def kernel(**inputs):
    nc, in_maps = build(inputs)
    from concourse.bass_utils import run_bass_kernel_spmd
    res = run_bass_kernel_spmd(nc, in_maps, list(range(NC)))
    outT = res.results[0]["out"]      # [4, B]
    return np.ascontiguousarray(outT.T.astype(F32))          # [B, 4]
